# revision 4
# baseline (speedup 1.0000x reference)
"""Trainium2 Bass kernel for nn_Conjunction_Shuffle.

Computes, for x (8192, 2048) f32 and W (2048, 1024) f32:

    out = (x * (x >= -1)) @ W + 0.1 * (1e-4 - |x| @ |W|)

Strategy:
  - Data-parallel: shard x along batch across 8 NeuronCores (1024 rows
    each), replicate W. No collectives needed.
  - Per core, both matmuls run on the TensorEngine in fp16 (inputs
    quantized fp32 -> fp16, exact fp32 accumulation in PSUM). Measured
    end-to-end scale-relative absmax error vs float64: ~1.6e-4.
  - x arrives batch-major; the contraction dim (IN) must sit on SBUF
    partitions, so x tiles are transposed on the PE (identity matmul),
    then the mask/abs elementwise ops run on DVE reading the transposed
    copy, emitting fp16 stationary tiles.
  - W is loaded in natural [IN, OUT] layout; fp16 cast on DVE, |W| on
    the scalar engine (Abs activation). The -0.1 factor is folded into
    the |x| stationary, and the +1e-5 constant is added during the
    PSUM -> SBUF copyback.
"""

import os
import tempfile
from contextlib import ExitStack

import numpy as np

import concourse.bass as bass
import concourse.mybir as mybir
import concourse.tile as tile
from concourse import bacc, bass_utils
from concourse.alu_op_type import AluOpType
from concourse.masks import make_identity

P = 128
B_FULL = 8192
IN = 2048
OUT = 1024
N_CORES = 8
B_SH = B_FULL // N_CORES  # 1024 rows per core

B_TILES = B_SH // P       # 8
K_TILES = IN // P         # 16
KG = 4                    # k-tiles per transpose/elementwise group
K_GROUPS = K_TILES // KG  # 4
N_FREE = 512              # matmul moving free dim (one PSUM bank)
N_TILES = OUT // N_FREE   # 2

F32 = mybir.dt.float32
F16 = mybir.dt.float16

DELTA = 0.1
MAX_ABS_W = 1e-4


def emit_body(ctx: ExitStack, tc, x_ap, w_ap, o_ap, pools):
    nc = tc.nc
    const_pool, wstage, xstage, xtpool, xmpool, psum_t, psum_mm, opool, resident = pools

    ident = const_pool.tile([P, P], F32, tag="ident")
    make_identity(nc, ident[:])

    # Resident fp16 moving-operand tiles (reused by every b-tile).
    wq = resident.tile([P, K_TILES, OUT], F16, tag="wq")    # fp16(W)
    wa = resident.tile([P, K_TILES, OUT], F16, tag="wa")    # fp16(-0.1*|W|)

    # ---- W preprocessing: chunks of KG k-tiles ----
    w_view = w_ap.rearrange("(k p) n -> p k n", p=P)  # [128, 16, 1024]
    for g in range(K_GROUPS):
        wf = wstage.tile([P, KG, OUT], F32, tag="wf")
        nc.sync.dma_start(wf[:], w_view[:, g * KG:(g + 1) * KG, :])
        ks = slice(g * KG, (g + 1) * KG)
        nc.vector.tensor_copy(wq[:, ks, :], wf[:])
        # |W| on ScalarE (abs has no DVE encoding on trn2), in place,
        # then -0.1*|W| -> fp16 on DVE.
        nc.scalar.activation(wf[:], wf[:], mybir.ActivationFunctionType.Abs)
        nc.vector.tensor_scalar(wa[:, ks, :], wf[:], -DELTA, None, AluOpType.mult)

    # ---- per b-tile: load, transpose on PE, mask/abs, matmuls ----
    for b in range(B_TILES):
        xb = xstage.tile([P, IN], F32, tag="xb")
        nc.sync.dma_start(xb[:], x_ap[b * P:(b + 1) * P, :])
        xm = xmpool.tile([P, K_TILES, P], F16, tag="xm")  # (x*(x>=-1)).T
        xa = xmpool.tile([P, K_TILES, P], F16, tag="xa")  # |x|.T
        for g in range(K_GROUPS):
            pst = psum_t.tile([P, KG, P], F32, tag="pst")
            for j in range(KG):
                k = g * KG + j
                nc.tensor.transpose(pst[:, j, :], xb[:, k * P:(k + 1) * P], ident[:])
            xt = xtpool.tile([P, KG, P], F32, tag="xt")
            nc.scalar.copy(xt[:], pst[:])
            ks = slice(g * KG, (g + 1) * KG)
            # xm = (xt >= -1) * xt  (one fused DVE op)
            nc.vector.scalar_tensor_tensor(
                xm[:, ks, :], xt[:], -1.0, xt[:],
                AluOpType.is_ge, AluOpType.mult,
            )
            # xa = |xt| on ScalarE
            nc.scalar.activation(xa[:, ks, :], xt[:],
                                 mybir.ActivationFunctionType.Abs)

        ob = opool.tile([P, OUT], F32, tag="ob")
        for n in range(N_TILES):
            pmm = psum_mm.tile([P, N_FREE], F32, tag="pmm")
            nsl = slice(n * N_FREE, (n + 1) * N_FREE)
            for k in range(K_TILES):
                nc.tensor.matmul(pmm[:], xm[:, k, :], wq[:, k, nsl],
                                 start=(k == 0), stop=False)
            for k in range(K_TILES):
                nc.tensor.matmul(pmm[:], xa[:, k, :], wa[:, k, nsl],
                                 start=False, stop=(k == K_TILES - 1))
            # out = acc + (1e-5)  [DELTA * MAX_ABS_W]
            nc.vector.tensor_scalar(ob[:, nsl], pmm[:], DELTA * MAX_ABS_W, None,
                                    AluOpType.add)
        nc.sync.dma_start(o_ap[b * P:(b + 1) * P, :], ob[:])


def build(repeats: int = 1):
    nc = bacc.Bacc("TRN2", target_bir_lowering=False, debug=False,
                   num_devices=N_CORES)
    x_ap = nc.dram_tensor("x", [B_SH, IN], F32, kind="ExternalInput").ap()
    w_ap = nc.dram_tensor("W", [IN, OUT], F32, kind="ExternalInput").ap()
    o_ap = nc.dram_tensor("out", [B_SH, OUT], F32, kind="ExternalOutput").ap()

    with tile.TileContext(nc) as tc, ExitStack() as ctx:
        pools = (
            ctx.enter_context(tc.tile_pool(name="const", bufs=1)),
            ctx.enter_context(tc.tile_pool(name="wstage", bufs=2)),
            ctx.enter_context(tc.tile_pool(name="xstage", bufs=2)),
            ctx.enter_context(tc.tile_pool(name="xt", bufs=3)),
            ctx.enter_context(tc.tile_pool(name="xm", bufs=3)),
            ctx.enter_context(tc.tile_pool(name="psum_t", bufs=2, space="PSUM")),
            ctx.enter_context(tc.tile_pool(name="psum_mm", bufs=4, space="PSUM")),
            ctx.enter_context(tc.tile_pool(name="opool", bufs=2)),
            ctx.enter_context(tc.tile_pool(name="resident", bufs=1)),
        )
        for _ in range(repeats):
            emit_body(ctx, tc, x_ap, w_ap, o_ap, pools)
    nc.compile()
    return nc


_cache: dict = {}


def _get(repeats: int = 1):
    if repeats not in _cache:
        _cache[repeats] = build(repeats)
    return _cache[repeats]


def run(x, W, repeats: int = 1):
    nc = _get(repeats)
    in_maps = [
        {"x": np.ascontiguousarray(x[c * B_SH:(c + 1) * B_SH]),
         "W": np.asarray(W)}
        for c in range(N_CORES)
    ]
    res = bass_utils.run_bass_kernel_spmd(nc, in_maps,
                                          core_ids=list(range(N_CORES)))
    out = np.concatenate([res.results[c]["out"] for c in range(N_CORES)], axis=0)
    return out


def kernel(x, W):
    return run(x, W, repeats=1)
